# revision 3
# baseline (speedup 1.0000x reference)
"""Chamfer loss Trainium2 kernel (data-parallel over batch, 8 NeuronCores).

Problem: x, y (8, 4096, 3) fp32; loss = mean_n [ mean_w min_v ||x_nv - y_nw||
+ mean_v min_w ||x_nv - y_nw|| ] (scalar fp32).

Per core (one batch):
  - Host packs augmented operands AX, AY [24, 4096] bf16 via an
    error-compensated 3-way hi/mid/lo split (products hh, hm, mh, hl, lh,
    mm + 3-way-split norm rows) so the PE gram matmul produces
    sq[v,w] = ||x_v||^2 + ||y_w||^2 - 2 x_v.y_w to ~1e-7 absolute
    accuracy while streaming at bf16 rate (1 col/cycle).
  - PE: 32 m-blocks x 8 matmuls of [24,128]^T @ [24,512] -> PSUM
    [128, 2048] groups (4 banks, double buffered).
  - ACT: evacuates PSUM -> SBUF fp16 (plain Copy) for 31 "A" m-blocks;
    ~115 us busy, the pipeline bottleneck.
  - DVE (~114 us busy, balanced with ACT):
      * col-direction running min: one fp16 2x tensor_tensor per m-block.
      * row-direction min: ONE 4x-mode tensor_scalar per m-block with
        fused accum_out (op1=min) -> rowmin[:, m] directly (the dual-output
        TensorScalarPtr reduces its own result at 4 elem/cycle).
      * 1 "C" m-block: DVE evacuates PSUM itself via 1x tensor_scalar with
        fused min-accum (evac + row-min in one pass), shifting one block of
        evacuation off ACT to balance the engines.
  - Epilogue: 32 PE transposes of the col accumulator into one PSUM tile;
    single 1x reduce-min -> per-w mins; clamp(max 0); one ACT sqrt with
    fused free-dim sum -> stot[128, 1].
  - Host: sum the 128 partials per core, scale by 1/V, average the 8
    per-core losses.
"""

import sys

sys.path.insert(0, "/opt/trn_rl_repo")

from contextlib import ExitStack

import ml_dtypes
import numpy as np

import concourse.bacc as bacc
import concourse.tile as tile
from concourse import mybir
from concourse.bass_utils import run_bass_kernel_spmd

BF16 = ml_dtypes.bfloat16

P = 128
V = 4096
KA = 24  # augmented contraction dim (3-way hi/mid/lo split)
NMM = 512  # matmul moving free dim (one fp32 PSUM bank)
GRP = 2048  # PSUM group (4 banks), double buffered
NG = V // GRP  # 2 groups per m-block
MB = V // P  # 32 m-blocks
C_BLOCKS = (24,)  # m-blocks evacuated by DVE (ACT<->DVE load balance)
BIG = 3.0e38  # finite +inf stand-in for min-identity scalars

_cache = {}


def _build_nc():
    F32 = mybir.dt.float32
    F16 = mybir.dt.float16
    mn = mybir.AluOpType.min
    X = mybir.AxisListType.X

    nc = bacc.Bacc("TRN2", target_bir_lowering=False)
    ax_d = nc.declare_dram_parameter("ax", [KA, V], mybir.dt.bfloat16, isOutput=False)
    ay_d = nc.declare_dram_parameter("ay", [KA, V], mybir.dt.bfloat16, isOutput=False)
    idh_d = nc.declare_dram_parameter("identh", [P, P], F16, isOutput=False)
    loss_d = nc.declare_dram_parameter("loss", [P, 1], F32, isOutput=True)

    with tile.TileContext(nc) as tc, ExitStack() as ctx:
        const = ctx.enter_context(tc.tile_pool(name="const", bufs=1))
        accs = ctx.enter_context(tc.tile_pool(name="accs", bufs=1))
        copies = ctx.enter_context(tc.tile_pool(name="copies", bufs=3))
        scratch = ctx.enter_context(tc.tile_pool(name="scratch", bufs=2))

        ax_sb = const.tile([KA, V], mybir.dt.bfloat16)
        ay_sb = const.tile([KA, V], mybir.dt.bfloat16)
        idh_sb = const.tile([P, P], F16)
        warmsrc = const.tile([1, 1], F32)
        warm = const.tile([1, 1], F32)
        nc.vector.memset(warmsrc[:], 1.0)
        nc.scalar.activation(warm[:], warmsrc[:], mybir.ActivationFunctionType.Sqrt)
        CH = V // 2
        for c in range(2):
            nc.sync.dma_start(ax_sb[:, c * CH : (c + 1) * CH], ax_d[:, c * CH : (c + 1) * CH])
            nc.sync.dma_start(ay_sb[:, c * CH : (c + 1) * CH], ay_d[:, c * CH : (c + 1) * CH])
        # identh is consumed only by the epilogue transposes ~110us later;
        # keep it off the critical path behind the ax/ay chunks
        nc.sync.dma_start(idh_sb[:], idh_d[:])

        cacc = accs.tile([P, V], F16, name="cacc")
        mins = accs.tile([P, 2 * MB], F32, name="mins")
        minsB = accs.tile([P, 2 * MB], F32, name="minsB")
        rowminC = accs.tile([P, 2 * len(C_BLOCKS)], F32, name="rowminC")
        rowmin = mins[:, :MB]
        colmin = mins[:, MB:]

        with tc.tile_pool(name="psum", bufs=2, space="PSUM") as psum:
            for m in range(MB):
                lhsT = ax_sb[:, m * P : (m + 1) * P]
                ct = copies.tile([P, V], F16, name="ct", tag="ct")
                is_c = m in C_BLOCKS
                for g in range(NG):
                    pst = psum.tile([P, GRP], F32, name=f"ps{g}", tag="ps")
                    for j in range(GRP // NMM):
                        c0 = g * GRP + j * NMM
                        nc.tensor.matmul(
                            pst[:, j * NMM : (j + 1) * NMM],
                            lhsT,
                            ay_sb[:, c0 : c0 + NMM],
                            start=True,
                            stop=True,
                        )
                    if is_c:
                        # DVE evacuation: fp32 PSUM -> fp16 copy, row-min
                        # partial fused into accum_out (1x mode).
                        ci = C_BLOCKS.index(m)
                        nc.vector.tensor_scalar(
                            out=ct[:, g * GRP : (g + 1) * GRP],
                            in0=pst[:],
                            scalar1=BIG,
                            scalar2=None,
                            op0=mn,
                            op1=mn,
                            accum_out=rowminC[:, 2 * ci + g : 2 * ci + g + 1],
                        )
                    else:
                        nc.scalar.copy(ct[:, g * GRP : (g + 1) * GRP], pst[:])

                # col-direction running min (one fp16 2x TT over [P, V])
                if m == 0:
                    nc.vector.tensor_copy(cacc[:], ct[:])
                else:
                    nc.vector.tensor_tensor(cacc[:], ct[:], cacc[:], mn)

                # row-direction min: single 4x tensor_scalar with fused
                # min-reduction into rowmin[:, m]
                if not is_c:
                    scr = scratch.tile([P, V], F16, name="scr", tag="scr")
                    nc.vector.tensor_scalar(
                        out=scr[:],
                        in0=ct[:],
                        scalar1=BIG,
                        scalar2=None,
                        op0=mn,
                        op1=mn,
                        accum_out=rowmin[:, m : m + 1],
                    )

        # Epilogue: transpose col accumulator -> per-w mins.
        with tc.tile_pool(name="psum_ep", bufs=1, space="PSUM") as psum_ep:
            tp = psum_ep.tile([P, V], F16, name="tp")
            for b in range(MB):
                nc.tensor.transpose(
                    tp[:, b * P : (b + 1) * P],
                    cacc[:, b * P : (b + 1) * P],
                    idh_sb[:],
                )
            nc.vector.tensor_reduce(
                colmin[:],
                tp[:].rearrange("p (a b) -> p a b", a=MB),
                axis=X,
                op=mn,
            )

            # merge the C-blocks' two group partials into their rowmin slot
            for ci, m in enumerate(C_BLOCKS):
                nc.vector.tensor_tensor(
                    rowmin[:, m : m + 1],
                    rowminC[:, 2 * ci : 2 * ci + 1],
                    rowminC[:, 2 * ci + 1 : 2 * ci + 2],
                    mn,
                )

            # clamp tiny negatives (rounding) then sqrt + fused free-dim sum
            nc.vector.tensor_scalar(
                out=minsB[:], in0=mins[:], scalar1=0.0, scalar2=None,
                op0=mybir.AluOpType.max,
            )
            stot = accs.tile([P, 1], F32, name="stot")
            nc.scalar.activation(
                minsB[:], minsB[:], mybir.ActivationFunctionType.Sqrt,
                accum_out=stot[:],
            )
            nc.sync.dma_start(loss_d[:], stot[:])

    nc.finalize()
    return nc


def _split3(v):
    """3-way bf16 split: v ~= h + m + l with residual ~2^-27 |v|."""
    f32 = np.float32
    h = v.astype(BF16)
    m = (v - h.astype(f32)).astype(BF16)
    l = (v - h.astype(f32) - m.astype(f32)).astype(BF16)
    return h, m, l


def _augment(x, y):
    """x, y: (V, 3) fp32 -> AX, AY [24, V] bf16 3-way-split gram operands.

    sq = x2 + y2 + x.(-2y); products kept: hh, hm, mh, hl, lh, mm
    (magnitude >= ~2^-16); x2/y2 carried as 3 bf16 rows each.
    """
    f32 = np.float32
    yy = (-2.0 * y).astype(f32)
    xh, xm, xl = _split3(x)
    yh, ym, yl = _split3(yy)
    x2 = np.einsum("vc,vc->v", x.astype(np.float64), x.astype(np.float64)).astype(f32)
    y2 = np.einsum("vc,vc->v", y.astype(np.float64), y.astype(np.float64)).astype(f32)
    x2h, x2m, x2l = _split3(x2)
    y2h, y2m, y2l = _split3(y2)
    one = np.ones(V, dtype=BF16)

    def cols(a):
        return [a[:, 0], a[:, 1], a[:, 2]]

    ax = np.stack(
        cols(xh) + cols(xh) + cols(xm) + cols(xh) + cols(xl) + cols(xm)
        + [x2h, x2m, x2l, one, one, one]
    )
    ay = np.stack(
        cols(yh) + cols(ym) + cols(yh) + cols(yl) + cols(yh) + cols(ym)
        + [one, one, one, y2h, y2m, y2l]
    )
    return ax, ay


def kernel(x, y):
    x = np.asarray(x, dtype=np.float32)
    y = np.asarray(y, dtype=np.float32)
    n = x.shape[0]
    assert x.shape == (n, V, 3) and y.shape == (n, V, 3) and n == 8

    if "nc" not in _cache:
        _cache["nc"] = _build_nc()
    nc = _cache["nc"]

    identh = np.eye(P, dtype=np.float16)
    in_maps = []
    for i in range(n):
        ax, ay = _augment(x[i], y[i])
        in_maps.append({"ax": ax, "ay": ay, "identh": identh})

    res = run_bass_kernel_spmd(
        nc, in_maps, list(range(n)), trace=_cache.get("trace", False)
    )
    _cache["last"] = res
    scale = 1.0 / V
    vals = [
        np.asarray(res.results[i]["loss"], dtype=np.float64).sum() * scale
        for i in range(n)
    ]
    return np.asarray(np.mean(vals), dtype=np.float32)


# revision 4
# speedup vs baseline: 1.3164x; 1.3164x over previous
"""Chamfer loss Trainium2 kernel (data-parallel over batch, 8 NeuronCores).

Problem: x, y (8, 4096, 3) fp32; loss = mean_n [ mean_w min_v ||x_nv - y_nw||
+ mean_v min_w ||x_nv - y_nw|| ] (scalar fp32).

Per core (one batch):
  - Host packs augmented operands AX, AY [24, 4096] bf16 via an
    error-compensated 3-way hi/mid/lo split (products hh, hm, mh, hl, lh,
    mm + 3-way-split norm rows) so the PE gram matmul produces
    sq[v,w] = ||x_v||^2 + ||y_w||^2 - 2 x_v.y_w to ~1e-7 absolute
    accuracy while streaming at bf16 rate.
  - PE: 32 m-blocks x 8 matmuls of [24,128]^T @ [24,512] -> PSUM
    [128, 2048] groups (4 banks, double buffered).
  - ACT: evacuates PSUM -> SBUF fp16 (plain Copy), 2 instructions per
    m-block (~122 us busy).
  - DVE (critical path ~156 us):
      * col-direction running min: one fp16 2x tensor_tensor per m-block.
      * row-direction: fold trees batched 4 m-blocks per instruction via
        3D access patterns (fold 4096->256 in 4 fp16 2x TTs over [P,4,*]
        views, then one strided tensor_reduce -> rowmin[:, 4k:4k+4]);
        batching quarters the per-instruction init/semaphore overhead.
  - Epilogue: 32 PE transposes of the col accumulator into one PSUM tile;
    single reduce-min -> per-w mins; clamp(max 0); one ACT sqrt with
    fused free-dim sum -> stot[128, 1].
  - Host: sum the 128 partials per core, scale by 1/V, average the 8
    per-core losses.
"""

import sys

sys.path.insert(0, "/opt/trn_rl_repo")

from contextlib import ExitStack

import ml_dtypes
import numpy as np

import concourse.bacc as bacc
import concourse.tile as tile
from concourse import mybir
from concourse.bass_utils import run_bass_kernel_spmd

BF16 = ml_dtypes.bfloat16

P = 128
V = 4096
KA = 24  # augmented contraction dim (3-way hi/mid/lo split)
NMM = 512  # matmul moving free dim (one fp32 PSUM bank)
GRP = 2048  # PSUM group (4 banks), double buffered
NG = V // GRP  # groups per m-block
MB = V // P  # 32 m-blocks
SB = 4  # m-blocks per DVE fold super-block

_cache = {}


def _build_nc():
    F32 = mybir.dt.float32
    F16 = mybir.dt.float16
    mn = mybir.AluOpType.min
    X = mybir.AxisListType.X

    nc = bacc.Bacc("TRN2", target_bir_lowering=False)
    ax_d = nc.declare_dram_parameter("ax", [KA, V], mybir.dt.bfloat16, isOutput=False)
    ay_d = nc.declare_dram_parameter("ay", [KA, V], mybir.dt.bfloat16, isOutput=False)
    idh_d = nc.declare_dram_parameter("identh", [P, P], F16, isOutput=False)
    loss_d = nc.declare_dram_parameter("loss", [P, 1], F32, isOutput=True)

    with tile.TileContext(nc) as tc, ExitStack() as ctx:
        const = ctx.enter_context(tc.tile_pool(name="const", bufs=1))
        accs = ctx.enter_context(tc.tile_pool(name="accs", bufs=1))
        copies = ctx.enter_context(tc.tile_pool(name="copies", bufs=2))
        scratch = ctx.enter_context(tc.tile_pool(name="scratch", bufs=2))

        ax_sb = const.tile([KA, V], mybir.dt.bfloat16)
        ay_sb = const.tile([KA, V], mybir.dt.bfloat16)
        idh_sb = const.tile([P, P], F16)
        warmsrc = const.tile([1, 1], F32)
        warm = const.tile([1, 1], F32)
        nc.vector.memset(warmsrc[:], 1.0)
        nc.scalar.activation(warm[:], warmsrc[:], mybir.ActivationFunctionType.Sqrt)
        CH = V // 2
        for c in range(2):
            nc.sync.dma_start(ax_sb[:, c * CH : (c + 1) * CH], ax_d[:, c * CH : (c + 1) * CH])
            nc.sync.dma_start(ay_sb[:, c * CH : (c + 1) * CH], ay_d[:, c * CH : (c + 1) * CH])
        # identh is consumed only by the epilogue transposes; keep it off
        # the critical path behind the ax/ay chunks
        nc.sync.dma_start(idh_sb[:], idh_d[:])

        cacc = accs.tile([P, V], F16, name="cacc")
        mins = accs.tile([P, 2 * MB], F32, name="mins")
        minsB = accs.tile([P, 2 * MB], F32, name="minsB")
        rowmin = mins[:, :MB]
        colmin = mins[:, MB:]

        with tc.tile_pool(name="psum", bufs=2, space="PSUM") as psum:
            for sb in range(MB // SB):
                # ct4 holds SB m-blocks' fp16 tiles contiguously so the row
                # fold tree can batch SB blocks per instruction
                ct4 = copies.tile([P, SB * V], F16, name="ct4", tag="ct4")
                for mi in range(SB):
                    m = sb * SB + mi
                    lhsT = ax_sb[:, m * P : (m + 1) * P]
                    ct = ct4[:, mi * V : (mi + 1) * V]
                    for g in range(NG):
                        pst = psum.tile([P, GRP], F32, name=f"ps{g}", tag="ps")
                        for j in range(GRP // NMM):
                            c0 = g * GRP + j * NMM
                            nc.tensor.matmul(
                                pst[:, j * NMM : (j + 1) * NMM],
                                lhsT,
                                ay_sb[:, c0 : c0 + NMM],
                                start=True,
                                stop=True,
                            )
                        nc.scalar.copy(ct[:, g * GRP : (g + 1) * GRP], pst[:])

                    # col-direction running min (one fp16 2x TT over [P, V])
                    if m == 0:
                        nc.vector.tensor_copy(cacc[:], ct[:])
                    else:
                        nc.vector.tensor_tensor(cacc[:], ct[:], cacc[:], mn)

                # row-direction fold tree, SB blocks per instruction:
                # 4096 -> 2048 -> 1024 -> 512 -> 256 -> strided reduce
                c3 = ct4[:].rearrange("p (m w) -> p m w", m=SB)
                H = V // 2
                scr = scratch.tile([P, SB * H], F16, name="scr", tag="scr")
                s3 = scr[:].rearrange("p (m w) -> p m w", m=SB)
                nc.vector.tensor_tensor(s3[:, :, :], c3[:, :, :H], c3[:, :, H:], mn)
                nc.vector.tensor_tensor(
                    s3[:, :, : H // 2], s3[:, :, : H // 2], s3[:, :, H // 2 :], mn
                )
                nc.vector.tensor_tensor(
                    s3[:, :, : H // 4], s3[:, :, : H // 4],
                    s3[:, :, H // 4 : H // 2], mn,
                )
                nc.vector.tensor_tensor(
                    s3[:, :, : H // 8], s3[:, :, : H // 8],
                    s3[:, :, H // 8 : H // 4], mn,
                )
                nc.vector.tensor_reduce(
                    rowmin[:, sb * SB : (sb + 1) * SB],
                    s3[:, :, : H // 8],
                    axis=X,
                    op=mn,
                )

        # Epilogue: transpose col accumulator -> per-w mins.
        with tc.tile_pool(name="psum_ep", bufs=1, space="PSUM") as psum_ep:
            tp = psum_ep.tile([P, V], F16, name="tp")
            for b in range(MB):
                nc.tensor.transpose(
                    tp[:, b * P : (b + 1) * P],
                    cacc[:, b * P : (b + 1) * P],
                    idh_sb[:],
                )
            nc.vector.tensor_reduce(
                colmin[:],
                tp[:].rearrange("p (a b) -> p a b", a=MB),
                axis=X,
                op=mn,
            )

            # clamp tiny negatives (rounding) then sqrt + fused free-dim sum
            nc.vector.tensor_scalar(
                out=minsB[:], in0=mins[:], scalar1=0.0, scalar2=None,
                op0=mybir.AluOpType.max,
            )
            stot = accs.tile([P, 1], F32, name="stot")
            nc.scalar.activation(
                minsB[:], minsB[:], mybir.ActivationFunctionType.Sqrt,
                accum_out=stot[:],
            )
            nc.sync.dma_start(loss_d[:], stot[:])

    nc.finalize()
    return nc


def _split3(v):
    """3-way bf16 split: v ~= h + m + l with residual ~2^-27 |v|."""
    f32 = np.float32
    h = v.astype(BF16)
    m = (v - h.astype(f32)).astype(BF16)
    l = (v - h.astype(f32) - m.astype(f32)).astype(BF16)
    return h, m, l


def _augment(x, y):
    """x, y: (V, 3) fp32 -> AX, AY [24, V] bf16 3-way-split gram operands.

    sq = x2 + y2 + x.(-2y); products kept: hh, hm, mh, hl, lh, mm
    (magnitude >= ~2^-16); x2/y2 carried as 3 bf16 rows each.
    """
    f32 = np.float32
    yy = (-2.0 * y).astype(f32)
    xh, xm, xl = _split3(x)
    yh, ym, yl = _split3(yy)
    x2 = np.einsum("vc,vc->v", x.astype(np.float64), x.astype(np.float64)).astype(f32)
    y2 = np.einsum("vc,vc->v", y.astype(np.float64), y.astype(np.float64)).astype(f32)
    x2h, x2m, x2l = _split3(x2)
    y2h, y2m, y2l = _split3(y2)
    one = np.ones(V, dtype=BF16)

    def cols(a):
        return [a[:, 0], a[:, 1], a[:, 2]]

    ax = np.stack(
        cols(xh) + cols(xh) + cols(xm) + cols(xh) + cols(xl) + cols(xm)
        + [x2h, x2m, x2l, one, one, one]
    )
    ay = np.stack(
        cols(yh) + cols(ym) + cols(yh) + cols(yl) + cols(yh) + cols(ym)
        + [one, one, one, y2h, y2m, y2l]
    )
    return ax, ay


def kernel(x, y):
    x = np.asarray(x, dtype=np.float32)
    y = np.asarray(y, dtype=np.float32)
    n = x.shape[0]
    assert x.shape == (n, V, 3) and y.shape == (n, V, 3) and n == 8

    if "nc" not in _cache:
        _cache["nc"] = _build_nc()
    nc = _cache["nc"]

    identh = np.eye(P, dtype=np.float16)
    in_maps = []
    for i in range(n):
        ax, ay = _augment(x[i], y[i])
        in_maps.append({"ax": ax, "ay": ay, "identh": identh})

    res = run_bass_kernel_spmd(
        nc, in_maps, list(range(n)), trace=_cache.get("trace", False)
    )
    _cache["last"] = res
    scale = 1.0 / V
    vals = [
        np.asarray(res.results[i]["loss"], dtype=np.float64).sum() * scale
        for i in range(n)
    ]
    return np.asarray(np.mean(vals), dtype=np.float32)
